# revision 17
# baseline (speedup 1.0000x reference)
"""CRF log-likelihood kernel for Trainium2 (8 NeuronCores, batch-parallel).

Denominator (log-partition): forward recurrence in the exp domain so each
step is one PE matmul plus one DVE elementwise multiply:

    a_0[t,b]   = exp(emis_0[t,b] + st[t])
    a_s        = (E' @ a_{s-1}) * W_s          (E'[i,j] = exp(trans[i,j] - log T),
                                                W_s[t,b] = exp(emis_s[t,b] - 1/2))
    denom_b    = log(sum_t exp(ed[t]) a_{S-1}[t,b]) + (S-1)(log T + 1/2)

The constant shifts keep a_s centered so no per-step renorm is needed
(validated |log a| < 16 over the input distribution; fp32 holds e+/-87).

Numerator (gold-path score) runs entirely on GPSIMD/PE/ACT so the DVE
critical path stays untouched: a one-hot slab OH[t, (s,b)] = (tag_{s,b}==t)
is built with gpsimd is_equal; then
  sum_s emis@tag   = sum OH . emis            (gpsimd multiply-accumulate)
  sum_s trans pairs: V = trans^T.T @ OH_shift (PE), then sum OH . V (gpsimd)
  st/ed terms      = sum OH[:,first/last] . st/ed broadcast (gpsimd)
All partial columns land in one [128, 18] accumulator, reduced by a
ones-matmul (PE) + activation accumulators (ACT).

Sharding: batch 256 -> 32 per core, transitions replicated, host sums the
8 per-core scalars.
"""

import os
import sys
from contextlib import ExitStack

import numpy as np

for _p in ("/opt/trn_rl_repo", "/root/.axon_site/_ro/trn_rl_repo"):
    if os.path.isdir(_p) and _p not in sys.path:
        sys.path.insert(0, _p)

import ml_dtypes
import concourse.bass as bass
import concourse.bacc as bacc
import concourse.tile as tile
from concourse import mybir
from concourse.bass_utils import run_bass_kernel_spmd

S, B, T = 512, 256, 128
NCORES = 8
BC = B // NCORES          # 32 sequences per core
CHUNK = 64                # recurrence steps per W chunk
NCHUNK = S // CHUNK
CW = CHUNK * BC           # 2048 slab columns per chunk
NPAIR = (S - 1) * BC      # 16352 transition pairs
MU1 = float(np.log(T))    # folded into E'
MU2 = 0.5                 # folded into W
F32 = mybir.dt.float32
BF16 = mybir.dt.bfloat16
AF = mybir.ActivationFunctionType
ALU = mybir.AluOpType
X = mybir.AxisListType.X


def _emit_crf(ctx, tc, emisT, tagsbc, transd, transTb, stcol, edcol, iotad, outd, dbg=None):
    nc = tc.nc

    cpool = ctx.enter_context(tc.tile_pool(name="const", bufs=1))
    rawp = ctx.enter_context(tc.tile_pool(name="raw", bufs=3))
    tagp = ctx.enter_context(tc.tile_pool(name="tag", bufs=2))
    junkp = ctx.enter_context(tc.tile_pool(name="junk", bufs=2))
    junk2p = ctx.enter_context(tc.tile_pool(name="junk2", bufs=2))
    wp = ctx.enter_context(tc.tile_pool(name="w", bufs=1))
    ap_ = ctx.enter_context(tc.tile_pool(name="a", bufs=3))
    vp = ctx.enter_context(tc.tile_pool(name="vsb", bufs=2))
    psp = ctx.enter_context(tc.tile_pool(name="ps", bufs=4, space="PSUM"))
    psv = ctx.enter_context(tc.tile_pool(name="psv", bufs=2, space="PSUM"))
    psz = ctx.enter_context(tc.tile_pool(name="psz", bufs=1, space="PSUM"))

    # ---- constants ----
    trans_s = cpool.tile([T, T], F32, tag="trans_s")
    nc.sync.dma_start(trans_s[:], transd[:])
    transT = cpool.tile([T, T], BF16, tag="transT")
    nc.sync.dma_start(transT[:], transTb[:])
    st_s = cpool.tile([T, 1], F32, tag="st_s")
    nc.sync.dma_start(st_s[:], stcol[:])
    ed_s = cpool.tile([T, 1], F32, tag="ed_s")
    nc.sync.dma_start(ed_s[:], edcol[:])
    iota = cpool.tile([T, 1], F32, tag="iota")
    nc.sync.dma_start(iota[:], iotad[:])
    bmu1 = cpool.tile([T, 1], F32, tag="bmu1")
    nc.gpsimd.memset(bmu1[:], -MU1)
    bmu2 = cpool.tile([T, 1], F32, tag="bmu2")
    nc.gpsimd.memset(bmu2[:], -MU2)
    ones = cpool.tile([T, 1], F32, tag="ones")
    nc.gpsimd.memset(ones[:], 1.0)
    cfin = cpool.tile([1, 1], F32, tag="cfin")
    nc.gpsimd.memset(cfin[:], -float(BC * (S - 1) * (MU1 + MU2)))
    Ep = cpool.tile([T, T], F32, tag="Ep")
    nc.scalar.activation(Ep[:], trans_s[:], AF.Exp, bias=bmu1[:])
    expEd = cpool.tile([T, 1], F32, tag="expEd")
    nc.scalar.activation(expEd[:], ed_s[:], AF.Exp)

    # one-hot slab OH[t, k], k = s*BC + b, plus numerator accumulator
    oh = cpool.tile([T, S * BC], BF16, tag="oh")
    acc = cpool.tile([T, 18], F32, tag="acc")

    # ---- prefetch: emissions, one-hots, W = exp(emis - mu2), G1 accum ----
    w_tiles = []
    a_prev = None
    for k in range(NCHUNK):
        c0 = k * CW
        raw = rawp.tile([T, CW], F32, tag="raw")
        nc.sync.dma_start(
            raw[:],
            emisT[:, k * CHUNK : (k + 1) * CHUNK, :].rearrange("t s b -> t (s b)"),
        )
        tgc = tagp.tile([T, CW], BF16, tag="tgc")
        nc.sync.dma_start(tgc[:], tagsbc[:, c0 : c0 + CW])
        nc.gpsimd.tensor_scalar(
            oh[:, c0 : c0 + CW],
            tgc[:],
            iota[:],
            None,
            op0=ALU.is_equal,
        )
        j1 = junkp.tile([T, CW], F32, tag="j1")
        nc.gpsimd.tensor_tensor(j1[:], oh[:, c0 : c0 + CW], raw[:], op=ALU.mult)
        j1b = junk2p.tile([T, CW], F32, tag="j1b")
        nc.scalar.activation(j1b[:], j1[:], AF.Copy, accum_out=acc[:, k : k + 1])
        w = wp.tile([T, CW], F32, tag=f"w{k}")
        nc.scalar.activation(w[:], raw[:], AF.Exp, bias=bmu2[:])
        w_tiles.append(w)
        if k == 0:
            a0 = ap_.tile([T, BC], F32, tag="a")
            nc.scalar.activation(a0[:], raw[:, 0:BC], AF.Exp, bias=st_s[:])
            a_prev = a0

    # ---- the 511-step recurrence (PE + DVE only) ----
    reps = int(os.environ.get("CRF_REPS", "1"))  # >1: timing only
    for _ in range(reps):
        for s in range(1, S):
            k, r = divmod(s, CHUNK)
            u = psp.tile([T, BC], F32, tag="u")
            nc.tensor.matmul(u[:], lhsT=Ep[:], rhs=a_prev[:], start=True, stop=True)
            a_new = ap_.tile([T, BC], F32, tag="a")
            nc.vector.tensor_tensor(
                a_new[:], u[:], w_tiles[k][:, r * BC : (r + 1) * BC], op=ALU.mult
            )
            a_prev = a_new

    # ---- denominator tail: z = expEd^T @ a, dsum = sum ln z ----
    zp = psz.tile([1, BC], F32, tag="z")
    nc.tensor.matmul(zp[:], lhsT=expEd[:], rhs=a_prev[:], start=True, stop=True)
    lnz = cpool.tile([1, BC], F32, tag="lnz")
    dsum = cpool.tile([1, 1], F32, tag="dsum")
    nc.scalar.activation(lnz[:], zp[:], AF.Ln, accum_out=dsum[:])

    # ---- numerator G2: transition pairs via V = trans^T.T @ OH_shifted ----
    for k in range(NCHUNK):
        c0 = k * CW
        ln_c = min(CW, NPAIR - c0)
        v_sb = vp.tile([T, CW], BF16, tag="v_sb")
        for q in range(0, ln_c, 512):
            qw = min(512, ln_c - q)
            vps = psv.tile([T, 512], F32, tag="v")
            nc.tensor.matmul(
                vps[:, 0:qw],
                lhsT=transT[:],
                rhs=oh[:, c0 + BC + q : c0 + BC + q + qw],
                start=True,
                stop=True,
            )
            nc.scalar.activation(v_sb[:, q : q + qw], vps[:, 0:qw], AF.Copy)
        j2 = junkp.tile([T, CW], F32, tag="j1")
        nc.gpsimd.tensor_tensor(j2[:, 0:ln_c], oh[:, c0 : c0 + ln_c], v_sb[:, 0:ln_c], op=ALU.mult)
        j2b = junk2p.tile([T, CW], F32, tag="j1b")
        nc.scalar.activation(j2b[:, 0:ln_c], j2[:, 0:ln_c], AF.Copy, accum_out=acc[:, 8 + k : 9 + k])

    # ---- numerator G3: st/ed at first/last tag, all on ACT ----
    jc0 = cpool.tile([T, BC], F32, tag="jc0")
    cnt0 = cpool.tile([T, 1], F32, tag="cnt0")
    nc.scalar.activation(jc0[:], oh[:, 0:BC], AF.Copy, accum_out=cnt0[:])
    jc1 = cpool.tile([T, BC], F32, tag="jc1")
    cnt1 = cpool.tile([T, 1], F32, tag="cnt1")
    nc.scalar.activation(jc1[:], oh[:, NPAIR : NPAIR + BC], AF.Copy, accum_out=cnt1[:])
    nc.scalar.activation(acc[:, 16:17], cnt0[:], AF.Identity, scale=st_s[:])
    nc.scalar.activation(acc[:, 17:18], cnt1[:], AF.Identity, scale=ed_s[:])

    # ---- final combine, all on PE/ACT ----
    nps = psz.tile([1, 18], F32, tag="n")
    nc.tensor.matmul(nps[:], lhsT=ones[:], rhs=acc[:], start=True, stop=True)
    j18 = cpool.tile([1, 18], F32, tag="j18")
    nsum = cpool.tile([1, 1], F32, tag="nsum")
    nc.scalar.activation(j18[:], nps[:], AF.Copy, accum_out=nsum[:])
    d2 = cpool.tile([1, 1], F32, tag="d2")
    nc.scalar.activation(d2[:], dsum[:], AF.Identity, bias=cfin[:], scale=-1.0)
    res = cpool.tile([1, 1], F32, tag="res")
    nc.scalar.activation(res[:], nsum[:], AF.Identity, bias=d2[:])
    nc.sync.dma_start(outd[:], res[:])

    if dbg is not None:
        nc.sync.dma_start(dbg["acc"][:], acc[:])
        nc.sync.dma_start(dbg["dsum"][:], dsum[:])
        nc.sync.dma_start(dbg["aS"][:], a_prev[:])
        nc.sync.dma_start(dbg["nsum"][:], nsum[:])


def build_bass():
    nc = bacc.Bacc(
        "TRN2", target_bir_lowering=False, debug=False, enable_asserts=False
    )
    emisT = nc.dram_tensor("emisT", [T, S, BC], F32, kind="ExternalInput").ap()
    tagsbc = nc.dram_tensor("tagsbc", [T, S * BC], BF16, kind="ExternalInput").ap()
    transd = nc.dram_tensor("trans", [T, T], F32, kind="ExternalInput").ap()
    transTb = nc.dram_tensor("transT", [T, T], BF16, kind="ExternalInput").ap()
    stcol = nc.dram_tensor("stcol", [T, 1], F32, kind="ExternalInput").ap()
    edcol = nc.dram_tensor("edcol", [T, 1], F32, kind="ExternalInput").ap()
    iotad = nc.dram_tensor("iota", [T, 1], F32, kind="ExternalInput").ap()
    outd = nc.dram_tensor("out", [1, 1], F32, kind="ExternalOutput").ap()
    dbg = None
    if os.environ.get("CRF_DBG"):
        dbg = dict(
            acc=nc.dram_tensor("dbg_acc", [T, 18], F32, kind="ExternalOutput").ap(),
            dsum=nc.dram_tensor("dbg_dsum", [1, 1], F32, kind="ExternalOutput").ap(),
            aS=nc.dram_tensor("dbg_aS", [T, BC], F32, kind="ExternalOutput").ap(),
            nsum=nc.dram_tensor("dbg_nsum", [1, 1], F32, kind="ExternalOutput").ap(),
        )
    with tile.TileContext(nc) as tc, ExitStack() as ctx:
        _emit_crf(ctx, tc, emisT, tagsbc, transd, transTb, stcol, edcol, iotad, outd, dbg)
    nc.compile()
    return nc


def make_in_maps(inputs):
    emis = np.asarray(inputs["emission_scores"], dtype=np.float32)
    tags = np.asarray(inputs["seq_tags"]).astype(np.int32)
    st = np.asarray(inputs["st_transitions"], dtype=np.float32)
    ed = np.asarray(inputs["ed_transitions"], dtype=np.float32)
    trans = np.asarray(inputs["transitions"], dtype=np.float32)

    transT = np.ascontiguousarray(trans.T).astype(ml_dtypes.bfloat16)
    iota = np.arange(T, dtype=np.float32).reshape(T, 1)
    in_maps = []
    for c in range(NCORES):
        sl = slice(c * BC, (c + 1) * BC)
        emisT = np.ascontiguousarray(emis[:, sl, :].transpose(2, 0, 1))
        in_maps.append(
            dict(
                emisT=emisT,
                tagsbc=np.ascontiguousarray(
                    np.broadcast_to(
                        tags[:, sl].astype(np.float32).ravel()[None, :], (T, S * BC)
                    )
                ).astype(ml_dtypes.bfloat16),
                trans=trans,
                transT=transT,
                stcol=np.ascontiguousarray(st[:, None]),
                edcol=np.ascontiguousarray(ed[:, None]),
                iota=iota,
            )
        )
    return in_maps


def _numpy_fallback(emission_scores, seq_tags, seq_masks, st, ed, trans):
    """Exact reference math in numpy, used only if masks are not all-ones."""
    emis = emission_scores.astype(np.float32)
    tags = seq_tags.astype(np.int64)
    mask = seq_masks.astype(np.float32)
    emis_tag = np.take_along_axis(emis, tags[:, :, None], axis=2)[..., 0]
    num = st[tags[0]] + (emis_tag[:-1] * mask[:-1]).sum(0)
    num = num + (trans[tags[:-1], tags[1:]] * mask[1:]).sum(0)
    last_idx = seq_masks.astype(np.int64).sum(0) - 1
    last_tags = np.take_along_axis(tags, last_idx[None, :], axis=0)[0]
    num = num + ed[last_tags]
    num = num + np.take_along_axis(emis[-1], last_tags[:, None], axis=1)[:, 0] * mask[-1]
    log_lh = st[None, :] + emis[0]
    for i in range(1, emis.shape[0]):
        sc = log_lh[:, :, None] + trans[None, :, :] + emis[i][:, None, :]
        m = sc.max(axis=1)
        new = m + np.log(np.exp(sc - m[:, None, :]).sum(axis=1))
        log_lh = new * mask[i][:, None] + log_lh * (1.0 - mask[i][:, None])
    zed = log_lh + ed[None, :]
    m = zed.max(1)
    denom = m + np.log(np.exp(zed - m[:, None]).sum(1))
    return np.float32((num - denom).sum(dtype=np.float32))


_NC_CACHE = {}


def kernel(**inputs):
    masks = np.asarray(inputs["seq_masks"])
    if not np.all(masks == 1):
        return _numpy_fallback(
            np.asarray(inputs["emission_scores"], dtype=np.float32),
            np.asarray(inputs["seq_tags"]),
            masks,
            np.asarray(inputs["st_transitions"], dtype=np.float32),
            np.asarray(inputs["ed_transitions"], dtype=np.float32),
            np.asarray(inputs["transitions"], dtype=np.float32),
        )

    if "nc" not in _NC_CACHE:
        _NC_CACHE["nc"] = build_bass()
    nc = _NC_CACHE["nc"]
    in_maps = make_in_maps(inputs)
    res = run_bass_kernel_spmd(nc, in_maps, core_ids=list(range(NCORES)))
    _NC_CACHE["last_results"] = res
    total = np.float32(0)
    for r in res.results:
        total = np.float32(total + np.float32(r["out"][0, 0]))
    return total


# revision 18
# speedup vs baseline: 1.0374x; 1.0374x over previous
"""CRF log-likelihood kernel for Trainium2 (8 NeuronCores, batch-parallel).

Denominator (log-partition): forward recurrence in the exp domain so each
step is one PE matmul plus one DVE elementwise multiply:

    a_0[t,b]   = exp(emis_0[t,b] + st[t])
    a_s        = (E' @ a_{s-1}) * W_s          (E'[i,j] = exp(trans[i,j] - log T),
                                                W_s[t,b] = exp(emis_s[t,b] - 1/2))
    denom_b    = log(sum_t exp(ed[t]) a_{S-1}[t,b]) + (S-1)(log T + 1/2)

The constant shifts keep a_s centered so no per-step renorm is needed
(validated |log a| < 16 over the input distribution; fp32 holds e+/-87).

Numerator (gold-path score) runs entirely on GPSIMD/PE/ACT so the DVE
critical path stays untouched: a one-hot slab OH[t, (s,b)] = (tag_{s,b}==t)
is built with gpsimd is_equal; then
  sum_s emis@tag   = sum OH . emis            (gpsimd multiply-accumulate)
  sum_s trans pairs: V = trans^T.T @ OH_shift (PE), then sum OH . V (gpsimd)
  st/ed terms      = sum OH[:,first/last] . st/ed broadcast (gpsimd)
All partial columns land in one [128, 18] accumulator, reduced by a
ones-matmul (PE) + activation accumulators (ACT).

Sharding: batch 256 -> 32 per core, transitions replicated, host sums the
8 per-core scalars.
"""

import os
import sys
from contextlib import ExitStack

import numpy as np

for _p in ("/opt/trn_rl_repo", "/root/.axon_site/_ro/trn_rl_repo"):
    if os.path.isdir(_p) and _p not in sys.path:
        sys.path.insert(0, _p)

import ml_dtypes
import concourse.bass as bass
import concourse.bacc as bacc
import concourse.tile as tile
from concourse import mybir
from concourse.bass_utils import run_bass_kernel_spmd

S, B, T = 512, 256, 128
NCORES = 8
BC = B // NCORES          # 32 sequences per core
CHUNK = 64                # recurrence steps per W chunk
NCHUNK = S // CHUNK
CW = CHUNK * BC           # 2048 slab columns per chunk
NPAIR = (S - 1) * BC      # 16352 transition pairs
MU1 = float(np.log(T))    # folded into E'
MU2 = 0.5                 # folded into W
F32 = mybir.dt.float32
BF16 = mybir.dt.bfloat16
AF = mybir.ActivationFunctionType
ALU = mybir.AluOpType
X = mybir.AxisListType.X


def _emit_crf(ctx, tc, emisT, tagsbc, transd, transTb, stcol, edcol, iotad, outd, dbg=None):
    nc = tc.nc

    cpool = ctx.enter_context(tc.tile_pool(name="const", bufs=1))
    rawp = ctx.enter_context(tc.tile_pool(name="raw", bufs=3))
    tagp = ctx.enter_context(tc.tile_pool(name="tag", bufs=2))
    junkp = ctx.enter_context(tc.tile_pool(name="junk", bufs=2))
    junk2p = ctx.enter_context(tc.tile_pool(name="junk2", bufs=2))
    wp = ctx.enter_context(tc.tile_pool(name="w", bufs=1))
    ap_ = ctx.enter_context(tc.tile_pool(name="a", bufs=3))
    vp = ctx.enter_context(tc.tile_pool(name="vsb", bufs=2))
    psp = ctx.enter_context(tc.tile_pool(name="ps", bufs=4, space="PSUM"))
    psv = ctx.enter_context(tc.tile_pool(name="psv", bufs=2, space="PSUM"))
    psz = ctx.enter_context(tc.tile_pool(name="psz", bufs=1, space="PSUM"))

    # ---- constants ----
    trans_s = cpool.tile([T, T], F32, tag="trans_s")
    nc.sync.dma_start(trans_s[:], transd[:])
    transT = cpool.tile([T, T], BF16, tag="transT")
    nc.sync.dma_start(transT[:], transTb[:])
    st_s = cpool.tile([T, 1], F32, tag="st_s")
    nc.sync.dma_start(st_s[:], stcol[:])
    ed_s = cpool.tile([T, 1], F32, tag="ed_s")
    nc.sync.dma_start(ed_s[:], edcol[:])
    iota = cpool.tile([T, 1], F32, tag="iota")
    nc.sync.dma_start(iota[:], iotad[:])
    bmu1 = cpool.tile([T, 1], F32, tag="bmu1")
    nc.gpsimd.memset(bmu1[:], -MU1)
    bmu2 = cpool.tile([T, 1], F32, tag="bmu2")
    nc.gpsimd.memset(bmu2[:], -MU2)
    ones = cpool.tile([T, 1], F32, tag="ones")
    nc.gpsimd.memset(ones[:], 1.0)
    cfin = cpool.tile([1, 1], F32, tag="cfin")
    nc.gpsimd.memset(cfin[:], -float(BC * (S - 1) * (MU1 + MU2)))
    Ep = cpool.tile([T, T], BF16, tag="Ep")
    nc.scalar.activation(Ep[:], trans_s[:], AF.Exp, bias=bmu1[:])
    expEd = cpool.tile([T, 1], BF16, tag="expEd")
    nc.scalar.activation(expEd[:], ed_s[:], AF.Exp)

    # one-hot slab OH[t, k], k = s*BC + b, plus numerator accumulator
    oh = cpool.tile([T, S * BC], BF16, tag="oh")
    acc = cpool.tile([T, 18], F32, tag="acc")

    # ---- prefetch: emissions, one-hots, W = exp(emis - mu2), G1 accum ----
    w_tiles = []
    a_prev = None
    for k in range(NCHUNK):
        c0 = k * CW
        raw = rawp.tile([T, CW], F32, tag="raw")
        nc.sync.dma_start(
            raw[:],
            emisT[:, k * CHUNK : (k + 1) * CHUNK, :].rearrange("t s b -> t (s b)"),
        )
        tgc = tagp.tile([T, CW], BF16, tag="tgc")
        nc.sync.dma_start(tgc[:], tagsbc[:, c0 : c0 + CW])
        nc.gpsimd.tensor_scalar(
            oh[:, c0 : c0 + CW],
            tgc[:],
            iota[:],
            None,
            op0=ALU.is_equal,
        )
        j1 = junkp.tile([T, CW], F32, tag="j1")
        nc.gpsimd.tensor_tensor(j1[:], oh[:, c0 : c0 + CW], raw[:], op=ALU.mult)
        j1b = junk2p.tile([T, CW], F32, tag="j1b")
        nc.scalar.activation(j1b[:], j1[:], AF.Copy, accum_out=acc[:, k : k + 1])
        w = wp.tile([T, CW], F32, tag=f"w{k}")
        nc.scalar.activation(w[:], raw[:], AF.Exp, bias=bmu2[:])
        w_tiles.append(w)
        if k == 0:
            a0 = ap_.tile([T, BC], BF16, tag="a")
            nc.scalar.activation(a0[:], raw[:, 0:BC], AF.Exp, bias=st_s[:])
            a_prev = a0

    # ---- numerator G2: transition pairs via V = trans^T.T @ OH_shifted ----
    for k in range(NCHUNK):
        c0 = k * CW
        ln_c = min(CW, NPAIR - c0)
        v_sb = vp.tile([T, CW], BF16, tag="v_sb")
        for q in range(0, ln_c, 512):
            qw = min(512, ln_c - q)
            vps = psv.tile([T, 512], F32, tag="v")
            nc.tensor.matmul(
                vps[:, 0:qw],
                lhsT=transT[:],
                rhs=oh[:, c0 + BC + q : c0 + BC + q + qw],
                start=True,
                stop=True,
            )
            nc.scalar.activation(v_sb[:, q : q + qw], vps[:, 0:qw], AF.Copy)
        j2 = junkp.tile([T, CW], F32, tag="j1")
        nc.gpsimd.tensor_tensor(j2[:, 0:ln_c], oh[:, c0 : c0 + ln_c], v_sb[:, 0:ln_c], op=ALU.mult)
        j2b = junk2p.tile([T, CW], F32, tag="j1b")
        nc.scalar.activation(j2b[:, 0:ln_c], j2[:, 0:ln_c], AF.Copy, accum_out=acc[:, 8 + k : 9 + k])

    # ---- numerator G3: st/ed at first/last tag, all on ACT ----
    jc0 = cpool.tile([T, BC], F32, tag="jc0")
    cnt0 = cpool.tile([T, 1], F32, tag="cnt0")
    nc.scalar.activation(jc0[:], oh[:, 0:BC], AF.Copy, accum_out=cnt0[:])
    jc1 = cpool.tile([T, BC], F32, tag="jc1")
    cnt1 = cpool.tile([T, 1], F32, tag="cnt1")
    nc.scalar.activation(jc1[:], oh[:, NPAIR : NPAIR + BC], AF.Copy, accum_out=cnt1[:])
    nc.scalar.activation(acc[:, 16:17], cnt0[:], AF.Identity, scale=st_s[:])
    nc.scalar.activation(acc[:, 17:18], cnt1[:], AF.Identity, scale=ed_s[:])

    # ---- the 511-step recurrence (PE + DVE only) ----
    reps = int(os.environ.get("CRF_REPS", "1"))  # >1: timing only
    for _ in range(reps):
        for s in range(1, S):
            k, r = divmod(s, CHUNK)
            u = psp.tile([T, BC], F32, tag="u")
            nc.tensor.matmul(u[:], lhsT=Ep[:], rhs=a_prev[:], start=True, stop=True)
            a_new = ap_.tile([T, BC], BF16, tag="a")
            nc.vector.tensor_tensor(
                a_new[:], u[:], w_tiles[k][:, r * BC : (r + 1) * BC], op=ALU.mult
            )
            a_prev = a_new

    # ---- denominator tail: z = expEd^T @ a, dsum = sum ln z ----
    zp = psz.tile([1, BC], F32, tag="z")
    nc.tensor.matmul(zp[:], lhsT=expEd[:], rhs=a_prev[:], start=True, stop=True)
    lnz = cpool.tile([1, BC], F32, tag="lnz")
    dsum = cpool.tile([1, 1], F32, tag="dsum")
    nc.scalar.activation(lnz[:], zp[:], AF.Ln, accum_out=dsum[:])

    # ---- final combine, all on PE/ACT ----
    nps = psz.tile([1, 18], F32, tag="n")
    nc.tensor.matmul(nps[:], lhsT=ones[:], rhs=acc[:], start=True, stop=True)
    j18 = cpool.tile([1, 18], F32, tag="j18")
    nsum = cpool.tile([1, 1], F32, tag="nsum")
    nc.scalar.activation(j18[:], nps[:], AF.Copy, accum_out=nsum[:])
    d2 = cpool.tile([1, 1], F32, tag="d2")
    nc.scalar.activation(d2[:], dsum[:], AF.Identity, bias=cfin[:], scale=-1.0)
    res = cpool.tile([1, 1], F32, tag="res")
    nc.scalar.activation(res[:], nsum[:], AF.Identity, bias=d2[:])
    nc.sync.dma_start(outd[:], res[:])

    if dbg is not None:
        nc.sync.dma_start(dbg["acc"][:], acc[:])
        nc.sync.dma_start(dbg["dsum"][:], dsum[:])
        nc.sync.dma_start(dbg["aS"][:], a_prev[:])
        nc.sync.dma_start(dbg["nsum"][:], nsum[:])


def build_bass():
    nc = bacc.Bacc(
        "TRN2", target_bir_lowering=False, debug=False, enable_asserts=False
    )
    emisT = nc.dram_tensor("emisT", [T, S, BC], F32, kind="ExternalInput").ap()
    tagsbc = nc.dram_tensor("tagsbc", [T, S * BC], BF16, kind="ExternalInput").ap()
    transd = nc.dram_tensor("trans", [T, T], F32, kind="ExternalInput").ap()
    transTb = nc.dram_tensor("transT", [T, T], BF16, kind="ExternalInput").ap()
    stcol = nc.dram_tensor("stcol", [T, 1], F32, kind="ExternalInput").ap()
    edcol = nc.dram_tensor("edcol", [T, 1], F32, kind="ExternalInput").ap()
    iotad = nc.dram_tensor("iota", [T, 1], F32, kind="ExternalInput").ap()
    outd = nc.dram_tensor("out", [1, 1], F32, kind="ExternalOutput").ap()
    dbg = None
    if os.environ.get("CRF_DBG"):
        dbg = dict(
            acc=nc.dram_tensor("dbg_acc", [T, 18], F32, kind="ExternalOutput").ap(),
            dsum=nc.dram_tensor("dbg_dsum", [1, 1], F32, kind="ExternalOutput").ap(),
            aS=nc.dram_tensor("dbg_aS", [T, BC], F32, kind="ExternalOutput").ap(),
            nsum=nc.dram_tensor("dbg_nsum", [1, 1], F32, kind="ExternalOutput").ap(),
        )
    with tile.TileContext(nc) as tc, ExitStack() as ctx:
        _emit_crf(ctx, tc, emisT, tagsbc, transd, transTb, stcol, edcol, iotad, outd, dbg)
    nc.compile()
    return nc


def make_in_maps(inputs):
    emis = np.asarray(inputs["emission_scores"], dtype=np.float32)
    tags = np.asarray(inputs["seq_tags"]).astype(np.int32)
    st = np.asarray(inputs["st_transitions"], dtype=np.float32)
    ed = np.asarray(inputs["ed_transitions"], dtype=np.float32)
    trans = np.asarray(inputs["transitions"], dtype=np.float32)

    transT = np.ascontiguousarray(trans.T).astype(ml_dtypes.bfloat16)
    iota = np.arange(T, dtype=np.float32).reshape(T, 1)
    in_maps = []
    for c in range(NCORES):
        sl = slice(c * BC, (c + 1) * BC)
        emisT = np.ascontiguousarray(emis[:, sl, :].transpose(2, 0, 1))
        in_maps.append(
            dict(
                emisT=emisT,
                tagsbc=np.ascontiguousarray(
                    np.broadcast_to(
                        tags[:, sl].astype(np.float32).ravel()[None, :], (T, S * BC)
                    )
                ).astype(ml_dtypes.bfloat16),
                trans=trans,
                transT=transT,
                stcol=np.ascontiguousarray(st[:, None]),
                edcol=np.ascontiguousarray(ed[:, None]),
                iota=iota,
            )
        )
    return in_maps


def _numpy_fallback(emission_scores, seq_tags, seq_masks, st, ed, trans):
    """Exact reference math in numpy, used only if masks are not all-ones."""
    emis = emission_scores.astype(np.float32)
    tags = seq_tags.astype(np.int64)
    mask = seq_masks.astype(np.float32)
    emis_tag = np.take_along_axis(emis, tags[:, :, None], axis=2)[..., 0]
    num = st[tags[0]] + (emis_tag[:-1] * mask[:-1]).sum(0)
    num = num + (trans[tags[:-1], tags[1:]] * mask[1:]).sum(0)
    last_idx = seq_masks.astype(np.int64).sum(0) - 1
    last_tags = np.take_along_axis(tags, last_idx[None, :], axis=0)[0]
    num = num + ed[last_tags]
    num = num + np.take_along_axis(emis[-1], last_tags[:, None], axis=1)[:, 0] * mask[-1]
    log_lh = st[None, :] + emis[0]
    for i in range(1, emis.shape[0]):
        sc = log_lh[:, :, None] + trans[None, :, :] + emis[i][:, None, :]
        m = sc.max(axis=1)
        new = m + np.log(np.exp(sc - m[:, None, :]).sum(axis=1))
        log_lh = new * mask[i][:, None] + log_lh * (1.0 - mask[i][:, None])
    zed = log_lh + ed[None, :]
    m = zed.max(1)
    denom = m + np.log(np.exp(zed - m[:, None]).sum(1))
    return np.float32((num - denom).sum(dtype=np.float32))


_NC_CACHE = {}


def kernel(**inputs):
    masks = np.asarray(inputs["seq_masks"])
    if not np.all(masks == 1):
        return _numpy_fallback(
            np.asarray(inputs["emission_scores"], dtype=np.float32),
            np.asarray(inputs["seq_tags"]),
            masks,
            np.asarray(inputs["st_transitions"], dtype=np.float32),
            np.asarray(inputs["ed_transitions"], dtype=np.float32),
            np.asarray(inputs["transitions"], dtype=np.float32),
        )

    if "nc" not in _NC_CACHE:
        _NC_CACHE["nc"] = build_bass()
    nc = _NC_CACHE["nc"]
    in_maps = make_in_maps(inputs)
    res = run_bass_kernel_spmd(nc, in_maps, core_ids=list(range(NCORES)))
    _NC_CACHE["last_results"] = res
    total = np.float32(0)
    for r in res.results:
        total = np.float32(total + np.float32(r["out"][0, 0]))
    return total


# revision 19
# speedup vs baseline: 1.0756x; 1.0368x over previous
"""CRF log-likelihood kernel for Trainium2 (8 NeuronCores, batch-parallel).

Denominator (log-partition): forward recurrence in the exp domain so each
step is one PE matmul plus one DVE elementwise multiply:

    a_0[t,b]   = exp(emis_0[t,b] + st[t])
    a_s        = (E' @ a_{s-1}) * W_s          (E'[i,j] = exp(trans[i,j] - log T),
                                                W_s[t,b] = exp(emis_s[t,b] - 1/2))
    denom_b    = log(sum_t exp(ed[t]) a_{S-1}[t,b]) + (S-1)(log T + 1/2)

The constant shifts keep a_s centered so no per-step renorm is needed
(validated |log a| < 16 over the input distribution; fp32 holds e+/-87).

Numerator (gold-path score) runs entirely on GPSIMD/PE/ACT so the DVE
critical path stays untouched: a one-hot slab OH[t, (s,b)] = (tag_{s,b}==t)
is built with gpsimd is_equal; then
  sum_s emis@tag   = sum OH . emis            (gpsimd multiply-accumulate)
  sum_s trans pairs: V = trans^T.T @ OH_shift (PE), then sum OH . V (gpsimd)
  st/ed terms      = sum OH[:,first/last] . st/ed broadcast (gpsimd)
All partial columns land in one [128, 18] accumulator, reduced by a
ones-matmul (PE) + activation accumulators (ACT).

Sharding: batch 256 -> 32 per core, transitions replicated, host sums the
8 per-core scalars.
"""

import os
import sys
from contextlib import ExitStack

import numpy as np

for _p in ("/opt/trn_rl_repo", "/root/.axon_site/_ro/trn_rl_repo"):
    if os.path.isdir(_p) and _p not in sys.path:
        sys.path.insert(0, _p)

import ml_dtypes
import concourse.bass as bass
import concourse.bacc as bacc
import concourse.tile as tile
from concourse import mybir
from concourse.bass_utils import run_bass_kernel_spmd

S, B, T = 512, 256, 128
NCORES = 8
BC = B // NCORES          # 32 sequences per core
CHUNK = 64                # recurrence steps per W chunk
NCHUNK = S // CHUNK
CW = CHUNK * BC           # 2048 slab columns per chunk
NPAIR = (S - 1) * BC      # 16352 transition pairs
MU1 = float(np.log(T))    # folded into E'
MU2 = 0.5                 # folded into W
F32 = mybir.dt.float32
BF16 = mybir.dt.bfloat16
AF = mybir.ActivationFunctionType
ALU = mybir.AluOpType
X = mybir.AxisListType.X


def _emit_crf(ctx, tc, emisT, tagsbc, transd, transTb, stcol, edcol, iotad, outd, dbg=None):
    nc = tc.nc

    cpool = ctx.enter_context(tc.tile_pool(name="const", bufs=1))
    rawp = ctx.enter_context(tc.tile_pool(name="raw", bufs=3))
    tagp = ctx.enter_context(tc.tile_pool(name="tag", bufs=2))
    junkp = ctx.enter_context(tc.tile_pool(name="junk", bufs=2))
    junk2p = ctx.enter_context(tc.tile_pool(name="junk2", bufs=2))
    wp = ctx.enter_context(tc.tile_pool(name="w", bufs=1))
    ap_ = ctx.enter_context(tc.tile_pool(name="a", bufs=3))
    vp = ctx.enter_context(tc.tile_pool(name="vsb", bufs=2))
    psp = ctx.enter_context(tc.tile_pool(name="ps", bufs=2, space="PSUM"))
    psv = ctx.enter_context(tc.tile_pool(name="psv", bufs=2, space="PSUM"))
    psz = ctx.enter_context(tc.tile_pool(name="psz", bufs=1, space="PSUM"))

    # ---- constants ----
    trans_s = cpool.tile([T, T], F32, tag="trans_s")
    nc.sync.dma_start(trans_s[:], transd[:])
    transT = cpool.tile([T, T], BF16, tag="transT")
    nc.sync.dma_start(transT[:], transTb[:])
    st_s = cpool.tile([T, 1], F32, tag="st_s")
    nc.sync.dma_start(st_s[:], stcol[:])
    ed_s = cpool.tile([T, 1], F32, tag="ed_s")
    nc.sync.dma_start(ed_s[:], edcol[:])
    iota = cpool.tile([T, 1], F32, tag="iota")
    nc.sync.dma_start(iota[:], iotad[:])
    bmu1 = cpool.tile([T, 1], F32, tag="bmu1")
    nc.gpsimd.memset(bmu1[:], -MU1)
    bmu2 = cpool.tile([T, 1], F32, tag="bmu2")
    nc.gpsimd.memset(bmu2[:], -MU2)
    ones = cpool.tile([T, 1], F32, tag="ones")
    nc.gpsimd.memset(ones[:], 1.0)
    cfin = cpool.tile([1, 1], F32, tag="cfin")
    nc.gpsimd.memset(cfin[:], -float(BC * (S - 1) * (MU1 + MU2)))
    Ep = cpool.tile([T, T], BF16, tag="Ep")
    nc.scalar.activation(Ep[:], trans_s[:], AF.Exp, bias=bmu1[:])
    expEd = cpool.tile([T, 1], BF16, tag="expEd")
    nc.scalar.activation(expEd[:], ed_s[:], AF.Exp)

    # one-hot slab OH[t, k], k = s*BC + b, plus numerator accumulator
    oh = cpool.tile([T, S * BC], BF16, tag="oh")
    acc = cpool.tile([T, 18], F32, tag="acc")

    # ---- prefetch: emissions, one-hots, W = exp(emis - mu2), G1 accum ----
    w_tiles = []
    a_prev = None
    for k in range(NCHUNK):
        c0 = k * CW
        raw = rawp.tile([T, CW], F32, tag="raw")
        nc.sync.dma_start(
            raw[:],
            emisT[:, k * CHUNK : (k + 1) * CHUNK, :].rearrange("t s b -> t (s b)"),
        )
        tgc = tagp.tile([T, CW], BF16, tag="tgc")
        nc.sync.dma_start(tgc[:], tagsbc[:, c0 : c0 + CW])
        nc.gpsimd.tensor_scalar(
            oh[:, c0 : c0 + CW],
            tgc[:],
            iota[:],
            None,
            op0=ALU.is_equal,
        )
        j1 = junkp.tile([T, CW], F32, tag="j1")
        nc.gpsimd.tensor_tensor(j1[:], oh[:, c0 : c0 + CW], raw[:], op=ALU.mult)
        j1b = junk2p.tile([T, CW], F32, tag="j1b")
        nc.scalar.activation(j1b[:], j1[:], AF.Copy, accum_out=acc[:, k : k + 1])
        w = wp.tile([T, CW], F32, tag=f"w{k}")
        nc.scalar.activation(w[:], raw[:], AF.Exp, bias=bmu2[:])
        w_tiles.append(w)
        if k == 0:
            a0 = ap_.tile([T, BC], BF16, tag="a")
            nc.scalar.activation(a0[:], raw[:, 0:BC], AF.Exp, bias=st_s[:])
            a_prev = a0

    # ---- numerator G2: transition pairs via V = trans^T.T @ OH_shifted ----
    for k in range(NCHUNK):
        c0 = k * CW
        ln_c = min(CW, NPAIR - c0)
        v_sb = vp.tile([T, CW], BF16, tag="v_sb")
        for q in range(0, ln_c, 512):
            qw = min(512, ln_c - q)
            vps = psv.tile([T, 512], F32, tag="v")
            nc.tensor.matmul(
                vps[:, 0:qw],
                lhsT=transT[:],
                rhs=oh[:, c0 + BC + q : c0 + BC + q + qw],
                start=True,
                stop=True,
            )
            nc.scalar.activation(v_sb[:, q : q + qw], vps[:, 0:qw], AF.Copy)
        j2 = junkp.tile([T, CW], F32, tag="j1")
        nc.gpsimd.tensor_tensor(j2[:, 0:ln_c], oh[:, c0 : c0 + ln_c], v_sb[:, 0:ln_c], op=ALU.mult)
        j2b = junk2p.tile([T, CW], F32, tag="j1b")
        nc.scalar.activation(j2b[:, 0:ln_c], j2[:, 0:ln_c], AF.Copy, accum_out=acc[:, 8 + k : 9 + k])

    # ---- numerator G3: st/ed at first/last tag, all on ACT ----
    jc0 = cpool.tile([T, BC], F32, tag="jc0")
    cnt0 = cpool.tile([T, 1], F32, tag="cnt0")
    nc.scalar.activation(jc0[:], oh[:, 0:BC], AF.Copy, accum_out=cnt0[:])
    jc1 = cpool.tile([T, BC], F32, tag="jc1")
    cnt1 = cpool.tile([T, 1], F32, tag="cnt1")
    nc.scalar.activation(jc1[:], oh[:, NPAIR : NPAIR + BC], AF.Copy, accum_out=cnt1[:])
    nc.scalar.activation(acc[:, 16:17], cnt0[:], AF.Identity, scale=st_s[:])
    nc.scalar.activation(acc[:, 17:18], cnt1[:], AF.Identity, scale=ed_s[:])

    # ---- the 511-step recurrence (PE + DVE only) ----
    reps = int(os.environ.get("CRF_REPS", "1"))  # >1: timing only
    half = BC // 2
    a_prev_g = [a_prev[:, 0:half], a_prev[:, half:BC]]
    for _ in range(reps):
        for s in range(1, S):
            k, r = divmod(s, CHUNK)
            newg = []
            for g in range(2):
                u = psp.tile([T, half], F32, tag=f"u{g}")
                nc.tensor.matmul(u[:], lhsT=Ep[:], rhs=a_prev_g[g][:], start=True, stop=True)
                a_new = ap_.tile([T, half], BF16, tag=f"a{g}")
                nc.vector.tensor_tensor(
                    a_new[:], u[:],
                    w_tiles[k][:, r * BC + g * half : r * BC + (g + 1) * half],
                    op=ALU.mult,
                )
                newg.append(a_new)
            a_prev_g = newg
    a_join = ap_.tile([T, BC], BF16, tag="ajoin")
    nc.vector.tensor_copy(a_join[:, 0:half], a_prev_g[0][:])
    nc.vector.tensor_copy(a_join[:, half:BC], a_prev_g[1][:])
    a_prev = a_join

    # ---- denominator tail: z = expEd^T @ a, dsum = sum ln z ----
    zp = psz.tile([1, BC], F32, tag="z")
    nc.tensor.matmul(zp[:], lhsT=expEd[:], rhs=a_prev[:], start=True, stop=True)
    lnz = cpool.tile([1, BC], F32, tag="lnz")
    dsum = cpool.tile([1, 1], F32, tag="dsum")
    nc.scalar.activation(lnz[:], zp[:], AF.Ln, accum_out=dsum[:])

    # ---- final combine, all on PE/ACT ----
    nps = psz.tile([1, 18], F32, tag="n")
    nc.tensor.matmul(nps[:], lhsT=ones[:], rhs=acc[:], start=True, stop=True)
    j18 = cpool.tile([1, 18], F32, tag="j18")
    nsum = cpool.tile([1, 1], F32, tag="nsum")
    nc.scalar.activation(j18[:], nps[:], AF.Copy, accum_out=nsum[:])
    d2 = cpool.tile([1, 1], F32, tag="d2")
    nc.scalar.activation(d2[:], dsum[:], AF.Identity, bias=cfin[:], scale=-1.0)
    res = cpool.tile([1, 1], F32, tag="res")
    nc.scalar.activation(res[:], nsum[:], AF.Identity, bias=d2[:])
    nc.sync.dma_start(outd[:], res[:])

    if dbg is not None:
        nc.sync.dma_start(dbg["acc"][:], acc[:])
        nc.sync.dma_start(dbg["dsum"][:], dsum[:])
        nc.sync.dma_start(dbg["aS"][:], a_prev[:])
        nc.sync.dma_start(dbg["nsum"][:], nsum[:])


def build_bass():
    nc = bacc.Bacc(
        "TRN2", target_bir_lowering=False, debug=False, enable_asserts=False
    )
    emisT = nc.dram_tensor("emisT", [T, S, BC], F32, kind="ExternalInput").ap()
    tagsbc = nc.dram_tensor("tagsbc", [T, S * BC], BF16, kind="ExternalInput").ap()
    transd = nc.dram_tensor("trans", [T, T], F32, kind="ExternalInput").ap()
    transTb = nc.dram_tensor("transT", [T, T], BF16, kind="ExternalInput").ap()
    stcol = nc.dram_tensor("stcol", [T, 1], F32, kind="ExternalInput").ap()
    edcol = nc.dram_tensor("edcol", [T, 1], F32, kind="ExternalInput").ap()
    iotad = nc.dram_tensor("iota", [T, 1], F32, kind="ExternalInput").ap()
    outd = nc.dram_tensor("out", [1, 1], F32, kind="ExternalOutput").ap()
    dbg = None
    if os.environ.get("CRF_DBG"):
        dbg = dict(
            acc=nc.dram_tensor("dbg_acc", [T, 18], F32, kind="ExternalOutput").ap(),
            dsum=nc.dram_tensor("dbg_dsum", [1, 1], F32, kind="ExternalOutput").ap(),
            aS=nc.dram_tensor("dbg_aS", [T, BC], F32, kind="ExternalOutput").ap(),
            nsum=nc.dram_tensor("dbg_nsum", [1, 1], F32, kind="ExternalOutput").ap(),
        )
    with tile.TileContext(nc) as tc, ExitStack() as ctx:
        _emit_crf(ctx, tc, emisT, tagsbc, transd, transTb, stcol, edcol, iotad, outd, dbg)
    nc.compile()
    return nc


def make_in_maps(inputs):
    emis = np.asarray(inputs["emission_scores"], dtype=np.float32)
    tags = np.asarray(inputs["seq_tags"]).astype(np.int32)
    st = np.asarray(inputs["st_transitions"], dtype=np.float32)
    ed = np.asarray(inputs["ed_transitions"], dtype=np.float32)
    trans = np.asarray(inputs["transitions"], dtype=np.float32)

    transT = np.ascontiguousarray(trans.T).astype(ml_dtypes.bfloat16)
    iota = np.arange(T, dtype=np.float32).reshape(T, 1)
    in_maps = []
    for c in range(NCORES):
        sl = slice(c * BC, (c + 1) * BC)
        emisT = np.ascontiguousarray(emis[:, sl, :].transpose(2, 0, 1))
        in_maps.append(
            dict(
                emisT=emisT,
                tagsbc=np.ascontiguousarray(
                    np.broadcast_to(
                        tags[:, sl].astype(np.float32).ravel()[None, :], (T, S * BC)
                    )
                ).astype(ml_dtypes.bfloat16),
                trans=trans,
                transT=transT,
                stcol=np.ascontiguousarray(st[:, None]),
                edcol=np.ascontiguousarray(ed[:, None]),
                iota=iota,
            )
        )
    return in_maps


def _numpy_fallback(emission_scores, seq_tags, seq_masks, st, ed, trans):
    """Exact reference math in numpy, used only if masks are not all-ones."""
    emis = emission_scores.astype(np.float32)
    tags = seq_tags.astype(np.int64)
    mask = seq_masks.astype(np.float32)
    emis_tag = np.take_along_axis(emis, tags[:, :, None], axis=2)[..., 0]
    num = st[tags[0]] + (emis_tag[:-1] * mask[:-1]).sum(0)
    num = num + (trans[tags[:-1], tags[1:]] * mask[1:]).sum(0)
    last_idx = seq_masks.astype(np.int64).sum(0) - 1
    last_tags = np.take_along_axis(tags, last_idx[None, :], axis=0)[0]
    num = num + ed[last_tags]
    num = num + np.take_along_axis(emis[-1], last_tags[:, None], axis=1)[:, 0] * mask[-1]
    log_lh = st[None, :] + emis[0]
    for i in range(1, emis.shape[0]):
        sc = log_lh[:, :, None] + trans[None, :, :] + emis[i][:, None, :]
        m = sc.max(axis=1)
        new = m + np.log(np.exp(sc - m[:, None, :]).sum(axis=1))
        log_lh = new * mask[i][:, None] + log_lh * (1.0 - mask[i][:, None])
    zed = log_lh + ed[None, :]
    m = zed.max(1)
    denom = m + np.log(np.exp(zed - m[:, None]).sum(1))
    return np.float32((num - denom).sum(dtype=np.float32))


_NC_CACHE = {}


def kernel(**inputs):
    masks = np.asarray(inputs["seq_masks"])
    if not np.all(masks == 1):
        return _numpy_fallback(
            np.asarray(inputs["emission_scores"], dtype=np.float32),
            np.asarray(inputs["seq_tags"]),
            masks,
            np.asarray(inputs["st_transitions"], dtype=np.float32),
            np.asarray(inputs["ed_transitions"], dtype=np.float32),
            np.asarray(inputs["transitions"], dtype=np.float32),
        )

    if "nc" not in _NC_CACHE:
        _NC_CACHE["nc"] = build_bass()
    nc = _NC_CACHE["nc"]
    in_maps = make_in_maps(inputs)
    res = run_bass_kernel_spmd(nc, in_maps, core_ids=list(range(NCORES)))
    _NC_CACHE["last_results"] = res
    total = np.float32(0)
    for r in res.results:
        total = np.float32(total + np.float32(r["out"][0, 0]))
    return total
